# revision 11
# baseline (speedup 1.0000x reference)
"""Trainium2 Bass kernel for nn_AverageItemProfile (scatter_memory).

Strategy: host-side routing groups interactions by item id and buckets items
into "run-length classes" (class r = items with exactly r interactions; rare
long runs are padded up to pow2 buckets >= 16). Within each on-chip chunk the
interactions are laid out l-major (slot-plane l holds the l-th interaction of
every item), so on device:
  - the scatter-add of ratings is an in-place tree of contiguous adds,
  - the running-average update (un-average, add, re-average, clip) is
    item-space elementwise math,
  - the gather back to interactions is a 0-step broadcast access pattern
    (GpSimd multiply),
  - the weighted sum is an innermost-8 reduction (Vector).
Everything on device is a regular strided op; the 8 NeuronCores each own
~1/8 of the items of every class (no collectives needed).
"""
import sys

sys.path.insert(0, "/opt/trn_rl_repo")

import numpy as np
from contextlib import ExitStack

import concourse.bass as bass
import concourse.tile as tile
from concourse import bacc, mybir
from concourse.bass_utils import run_bass_kernel_spmd

P = 128            # partitions
NC = 8             # cores
A = 8              # aspects
MIN_R, MAX_R = 1.0, 5.0
TILE_COLS = 512    # target stream columns per on-chip chunk
MERGE_FROM = 9     # classes with r >= this are padded into pow2 buckets
F32 = mybir.dt.float32

_program_cache = {}


def _bucket_of(r):
    """Slot-count bucket for run length r (identity below MERGE_FROM)."""
    r = np.asarray(r)
    m = r >= MERGE_FROM
    if not np.any(m):
        return r.copy()
    p2 = 2 ** np.ceil(np.log2(np.maximum(r, 1))).astype(np.int64)
    return np.where(m, np.maximum(p2, 16), r)


def _chunks_of(layout):
    """Split the class layout into on-chip chunks of <= TILE_COLS stream
    columns. Returns [(cls_index, q_start, r, nq, scol0, icol0, merged)].
    Each chunk is its own l-major block of nq items x r slot-planes."""
    chunks = []
    icol = 0
    scol = 0
    for ci, (r, q, merged) in enumerate(layout):
        q_step = max(1, TILE_COLS // r)
        done = 0
        while done < q:
            nq = min(q_step, q - done)
            chunks.append((ci, done, r, nq, scol, icol, merged))
            icol += nq
            scol += nq * r
            done += nq
    return chunks


def _schedule_order(chunks):
    """Emission order: big chunks first, with small (latency-bound) chunks
    interleaved so their serial dependency chains hide under big chunks'
    DMA/compute. Offsets are baked per chunk, so order is free to choose."""
    big = sorted([c for c in chunks if c[3] * c[2] >= 256], key=lambda c: -c[3] * c[2])
    small = [c for c in chunks if c[3] * c[2] < 256]
    out = []
    si = 0
    for i, b in enumerate(big):
        out.append(b)
        if i >= 1 and si < len(small):
            out.append(small[si])
            si += 1
    out.extend(small[si:])
    return out


def _route(I_ids, n_rows):
    """Host routing: group interactions by item, bucket items by run length,
    assign items round-robin to (core, partition, item-col), and map every
    interaction to its (core, partition, stream-col) slot.

    Touches only the int index tensor; all float math stays on device.
    """
    B = I_ids.shape[0]
    ids = I_ids.astype(np.int64)
    counts = np.bincount(ids, minlength=n_rows)
    r_int = counts[ids]
    bkt_int = _bucket_of(r_int)
    order = np.argsort((bkt_int << 21) | ids, kind="stable")
    sid = ids[order]
    new_item = np.empty(B, np.bool_)
    new_item[0] = True
    new_item[1:] = sid[1:] != sid[:-1]
    jglob = np.cumsum(new_item) - 1                       # item rank per interaction
    run_start = np.flatnonzero(new_item)
    l_in_run = np.arange(B) - run_start[jglob]

    item_ids = sid[new_item]                              # distinct ids, (bkt, id) order
    item_r = counts[item_ids].astype(np.int64)
    item_bkt = _bucket_of(item_r)
    rvals, k_list = np.unique(item_bkt, return_counts=True)

    q_list = -(-k_list // (NC * P))                       # item cols per class
    icol_off = np.concatenate([[0], np.cumsum(q_list)])
    merged = rvals >= MERGE_FROM
    layout = tuple(zip(rvals.tolist(), q_list.tolist(), merged.tolist()))

    cls_idx = np.searchsorted(rvals, item_bkt)
    cls_first = np.concatenate([[0], np.cumsum(k_list)])
    jc = np.arange(item_ids.shape[0]) - cls_first[cls_idx]
    core_j = jc % NC
    p_j = (jc // NC) % P
    q_j = jc // (NC * P)
    item_col = icol_off[cls_idx] + q_j

    # chunk-aware l-major slot mapping: item (cls, q_j) -> slot_base/stride
    ncls = len(layout)
    max_q = int(q_list.max())
    base_of = np.zeros((ncls, max_q), np.int64)
    strd_of = np.zeros((ncls, max_q), np.int64)
    tot_cols = 0
    for ci, q0, r, nq, scol0, icol0, mrg in _chunks_of(layout):
        base_of[ci, q0:q0 + nq] = scol0 + np.arange(nq)
        strd_of[ci, q0:q0 + nq] = nq
        tot_cols = max(tot_cols, scol0 + nq * r)
    slot_base = base_of[cls_idx, q_j]
    slot_stride = strd_of[cls_idx, q_j]

    core_t = core_j[jglob]
    p_t = p_j[jglob]
    col_t = slot_base[jglob] + l_in_run * slot_stride[jglob]

    return dict(
        order=order, item_ids=item_ids, item_r=item_r,
        core_j=core_j, p_j=p_j, item_col=item_col,
        core_t=core_t, p_t=p_t, col_t=col_t,
        layout=layout,
        tot_icols=int(icol_off[-1]), tot_cols=int(tot_cols),
    )


def _build_program(layout, tot_icols, tot_cols):
    nc = bacc.Bacc("TRN2", debug=False)
    d_rat = nc.dram_tensor("rat_s", [P, tot_cols * A], F32, kind="ExternalInput")
    d_wts = nc.dram_tensor("wts_s", [P, tot_cols * A], F32, kind="ExternalInput")
    d_par = nc.dram_tensor("par_s", [P, tot_icols * A], F32, kind="ExternalInput")
    d_cnt = nc.dram_tensor("cnt_s", [P, tot_icols], F32, kind="ExternalInput")
    d_n = nc.dram_tensor("n_s", [P, tot_icols], F32, kind="ExternalInput")
    d_out = nc.dram_tensor("preds_s", [P, tot_cols], F32, kind="ExternalOutput")

    with tile.TileContext(nc) as tc, ExitStack() as ctx:
        in_pool = ctx.enter_context(tc.tile_pool(name="in", bufs=4))
        it_pool = ctx.enter_context(tc.tile_pool(name="it", bufs=1))
        out_pool = ctx.enter_context(tc.tile_pool(name="out", bufs=3))

        # one-time item-space prepass over ALL items:
        #   pc_all = par * cnt (per-aspect broadcast), recip_all = 1/max(cnt+n,1)
        par_all = it_pool.tile([P, tot_icols * A], F32, tag="par")
        nc.sync.dma_start(par_all[:], d_par[:, :])
        cnt_all = it_pool.tile([P, tot_icols], F32, tag="cnt")
        nc.sync.dma_start(cnt_all[:], d_cnt[:, :])
        n_all = it_pool.tile([P, tot_icols], F32, tag="nn")
        nc.sync.dma_start(n_all[:], d_n[:, :])
        recip_all = it_pool.tile([P, tot_icols], F32, tag="recip")
        nc.vector.tensor_tensor(recip_all[:], cnt_all[:], n_all[:],
                                op=mybir.AluOpType.add)
        nc.vector.tensor_scalar_max(recip_all[:], recip_all[:], 1.0)
        nc.vector.reciprocal(recip_all[:], recip_all[:])
        nc.vector.tensor_tensor(
            par_all[:].rearrange("p (q a) -> p q a", a=A),
            par_all[:].rearrange("p (q a) -> p q a", a=A),
            cnt_all[:].unsqueeze(2).broadcast_to([P, tot_icols, A]),
            op=mybir.AluOpType.mult)
        pc_all = par_all

        for ci, q0, r, nq, scol0, icol0, merged in _schedule_order(_chunks_of(layout)):
            m = nq * r
            w = nq * A                                    # elements per slot-plane
            rat = in_pool.tile([P, m * A], F32, tag="rat")
            nc.sync.dma_start(rat[:], d_rat[:, scol0 * A:(scol0 + m) * A])
            wts = in_pool.tile([P, m * A], F32, tag="wts")
            nc.scalar.dma_start(wts[:], d_wts[:, scol0 * A:(scol0 + m) * A])

            # segment sums: in-place tree of contiguous adds over slot-planes
            k = r
            while k > 1:
                h = k // 2
                nc.vector.tensor_tensor(
                    rat[:, 0:h * w], rat[:, 0:h * w],
                    rat[:, (k - h) * w:k * w], op=mybir.AluOpType.add)
                k -= h
            ssum = rat[:, 0:w]

            # work = clip((pc + ssum) * recip, 1, 5), in place in pc_all
            work = pc_all[:, icol0 * A:(icol0 + nq) * A]
            nc.vector.tensor_tensor(work, work, ssum, op=mybir.AluOpType.add)
            nc.vector.tensor_tensor(
                work.rearrange("p (q a) -> p q a", a=A),
                work.rearrange("p (q a) -> p q a", a=A),
                recip_all[:, icol0:icol0 + nq]
                    .unsqueeze(2).broadcast_to([P, nq, A]),
                op=mybir.AluOpType.mult)
            nc.vector.tensor_scalar(work, work, MIN_R, MAX_R,
                                    op0=mybir.AluOpType.max,
                                    op1=mybir.AluOpType.min)

            # wts *= work (broadcast profiles over slot-planes)
            if r > 1:
                nc.vector.tensor_tensor(
                    wts[:].rearrange("p (r qa) -> p r qa", r=r),
                    wts[:].rearrange("p (r qa) -> p r qa", r=r),
                    work.unsqueeze(1).broadcast_to([P, r, w]),
                    op=mybir.AluOpType.mult)
            else:
                nc.vector.tensor_tensor(wts[:], wts[:], work,
                                        op=mybir.AluOpType.mult)

            pred = out_pool.tile([P, m], F32, tag="pred")
            nc.vector.tensor_reduce(
                pred[:].rearrange("p (lq) -> p lq", lq=m),
                wts[:].rearrange("p (lq a) -> p lq a", lq=m, a=A),
                axis=mybir.AxisListType.X, op=mybir.AluOpType.add)
            nc.scalar.dma_start(d_out[:, scol0:scol0 + m], pred[:])

    nc.compile()
    return nc


def _get_program(layout, tot_icols, tot_cols):
    key = (layout, tot_icols, tot_cols)
    if key not in _program_cache:
        _program_cache[key] = _build_program(layout, tot_icols, tot_cols)
    return _program_cache[key]


def _prepare(items_parameters, items_counters, I_ids, A_weights, A_ratings):
    rt = _route(np.asarray(I_ids), items_parameters.shape[0])
    ti, tc_ = rt["tot_icols"], rt["tot_cols"]

    rat_s = np.zeros((NC, P, tc_, A), np.float32)
    wts_s = np.zeros((NC, P, tc_, A), np.float32)
    rat_s[rt["core_t"], rt["p_t"], rt["col_t"]] = np.asarray(A_ratings)[rt["order"]]
    wts_s[rt["core_t"], rt["p_t"], rt["col_t"]] = np.asarray(A_weights)[rt["order"]]
    par_s = np.zeros((NC, P, ti, A), np.float32)
    cnt_s = np.zeros((NC, P, ti), np.float32)
    n_s = np.zeros((NC, P, ti), np.float32)
    par_s[rt["core_j"], rt["p_j"], rt["item_col"]] = np.asarray(items_parameters)[rt["item_ids"]]
    cnt_s[rt["core_j"], rt["p_j"], rt["item_col"]] = np.asarray(items_counters)[rt["item_ids"]]
    n_s[rt["core_j"], rt["p_j"], rt["item_col"]] = rt["item_r"].astype(np.float32)

    in_maps = [dict(rat_s=rat_s[c].reshape(P, tc_ * A),
                    wts_s=wts_s[c].reshape(P, tc_ * A),
                    par_s=par_s[c].reshape(P, ti * A),
                    cnt_s=cnt_s[c].reshape(P, ti),
                    n_s=n_s[c].reshape(P, ti))
               for c in range(NC)]
    return rt, in_maps


def _run(inputs, trace=False, **kw):
    rt, in_maps = _prepare(**inputs)
    nc = _get_program(rt["layout"], rt["tot_icols"], rt["tot_cols"])
    res = run_bass_kernel_spmd(nc, in_maps, core_ids=list(range(NC)),
                               trace=trace, **kw)
    preds_s = np.stack([res.results[c]["preds_s"] for c in range(NC)])
    B = rt["order"].shape[0]
    out = np.empty(B, np.float32)
    out[rt["order"]] = preds_s[rt["core_t"], rt["p_t"], rt["col_t"]]
    return out, res


def kernel(items_parameters, items_counters, I_ids, A_weights, A_ratings):
    out, _ = _run(dict(items_parameters=items_parameters,
                       items_counters=items_counters,
                       I_ids=I_ids,
                       A_weights=A_weights,
                       A_ratings=A_ratings))
    return out


# revision 12
# speedup vs baseline: 1.0687x; 1.0687x over previous
"""Trainium2 Bass kernel for nn_AverageItemProfile (scatter_memory).

Strategy: host-side routing groups interactions by item id and buckets items
into "run-length classes" (class r = items with exactly r interactions; rare
long runs are padded up to pow2 buckets >= 16). Within each on-chip chunk the
interactions are laid out l-major (slot-plane l holds the l-th interaction of
every item), so on device:
  - the scatter-add of ratings is an in-place tree of contiguous adds,
  - the running-average update (un-average, add, re-average, clip) is
    item-space elementwise math,
  - the gather back to interactions is a 0-step broadcast access pattern
    (GpSimd multiply),
  - the weighted sum is an innermost-8 reduction (Vector).
Everything on device is a regular strided op; the 8 NeuronCores each own
~1/8 of the items of every class (no collectives needed).
"""
import sys

sys.path.insert(0, "/opt/trn_rl_repo")

import numpy as np
from contextlib import ExitStack

import concourse.bass as bass
import concourse.tile as tile
from concourse import bacc, mybir
from concourse.bass_utils import run_bass_kernel_spmd

P = 128            # partitions
NC = 8             # cores
A = 8              # aspects
MIN_R, MAX_R = 1.0, 5.0
TILE_COLS = 512    # target stream columns per on-chip chunk
MERGE_FROM = 9     # classes with r >= this are padded into pow2 buckets
F32 = mybir.dt.float32

_program_cache = {}


def _bucket_of(r):
    """Slot-count bucket for run length r (identity below MERGE_FROM)."""
    r = np.asarray(r)
    m = r >= MERGE_FROM
    if not np.any(m):
        return r.copy()
    p2 = 2 ** np.ceil(np.log2(np.maximum(r, 1))).astype(np.int64)
    return np.where(m, np.maximum(p2, 16), r)


def _chunks_of(layout):
    """Split the class layout into on-chip chunks of <= TILE_COLS stream
    columns. Returns [(cls_index, q_start, r, nq, scol0, icol0, merged)].
    Each chunk is its own l-major block of nq items x r slot-planes."""
    chunks = []
    icol = 0
    scol = 0
    for ci, (r, q, merged) in enumerate(layout):
        q_step = max(1, TILE_COLS // r)
        done = 0
        while done < q:
            nq = min(q_step, q - done)
            chunks.append((ci, done, r, nq, scol, icol, merged))
            icol += nq
            scol += nq * r
            done += nq
    return chunks


def _schedule_order(chunks):
    """Emission order: big chunks first, with small (latency-bound) chunks
    interleaved so their serial dependency chains hide under big chunks'
    DMA/compute. Offsets are baked per chunk, so order is free to choose."""
    big = sorted([c for c in chunks if c[3] * c[2] >= 256], key=lambda c: -c[3] * c[2])
    small = [c for c in chunks if c[3] * c[2] < 256]
    out = []
    si = 0
    for i, b in enumerate(big):
        out.append(b)
        if i >= 1 and si < len(small):
            out.append(small[si])
            si += 1
    out.extend(small[si:])
    return out


def _route(I_ids, n_rows):
    """Host routing: group interactions by item, bucket items by run length,
    assign items round-robin to (core, partition, item-col), and map every
    interaction to its (core, partition, stream-col) slot.

    Touches only the int index tensor; all float math stays on device.
    """
    B = I_ids.shape[0]
    ids = I_ids.astype(np.int64)
    counts = np.bincount(ids, minlength=n_rows)
    r_int = counts[ids]
    bkt_int = _bucket_of(r_int)
    order = np.argsort((bkt_int << 21) | ids, kind="stable")
    sid = ids[order]
    new_item = np.empty(B, np.bool_)
    new_item[0] = True
    new_item[1:] = sid[1:] != sid[:-1]
    jglob = np.cumsum(new_item) - 1                       # item rank per interaction
    run_start = np.flatnonzero(new_item)
    l_in_run = np.arange(B) - run_start[jglob]

    item_ids = sid[new_item]                              # distinct ids, (bkt, id) order
    item_r = counts[item_ids].astype(np.int64)
    item_bkt = _bucket_of(item_r)
    rvals, k_list = np.unique(item_bkt, return_counts=True)

    q_list = -(-k_list // (NC * P))                       # item cols per class
    icol_off = np.concatenate([[0], np.cumsum(q_list)])
    merged = rvals >= MERGE_FROM
    layout = tuple(zip(rvals.tolist(), q_list.tolist(), merged.tolist()))

    cls_idx = np.searchsorted(rvals, item_bkt)
    cls_first = np.concatenate([[0], np.cumsum(k_list)])
    jc = np.arange(item_ids.shape[0]) - cls_first[cls_idx]
    core_j = jc % NC
    p_j = (jc // NC) % P
    q_j = jc // (NC * P)
    item_col = icol_off[cls_idx] + q_j

    # chunk-aware l-major slot mapping: item (cls, q_j) -> slot_base/stride
    ncls = len(layout)
    max_q = int(q_list.max())
    base_of = np.zeros((ncls, max_q), np.int64)
    strd_of = np.zeros((ncls, max_q), np.int64)
    tot_cols = 0
    for ci, q0, r, nq, scol0, icol0, mrg in _chunks_of(layout):
        base_of[ci, q0:q0 + nq] = scol0 + np.arange(nq)
        strd_of[ci, q0:q0 + nq] = nq
        tot_cols = max(tot_cols, scol0 + nq * r)
    slot_base = base_of[cls_idx, q_j]
    slot_stride = strd_of[cls_idx, q_j]

    core_t = core_j[jglob]
    p_t = p_j[jglob]
    col_t = slot_base[jglob] + l_in_run * slot_stride[jglob]

    return dict(
        order=order, item_ids=item_ids, item_r=item_r,
        core_j=core_j, p_j=p_j, item_col=item_col,
        core_t=core_t, p_t=p_t, col_t=col_t,
        layout=layout,
        tot_icols=int(icol_off[-1]), tot_cols=int(tot_cols),
    )


def _build_program(layout, tot_icols, tot_cols):
    nc = bacc.Bacc("TRN2", debug=False)
    d_rat = nc.dram_tensor("rat_s", [P, tot_cols * A], F32, kind="ExternalInput")
    d_wts = nc.dram_tensor("wts_s", [P, tot_cols * A], F32, kind="ExternalInput")
    d_par = nc.dram_tensor("par_s", [P, tot_icols * A], F32, kind="ExternalInput")
    d_cnt = nc.dram_tensor("cnt_s", [P, tot_icols], F32, kind="ExternalInput")
    d_n = nc.dram_tensor("n_s", [P, tot_icols], F32, kind="ExternalInput")
    d_out = nc.dram_tensor("preds_s", [P, tot_cols], F32, kind="ExternalOutput")

    with tile.TileContext(nc) as tc, ExitStack() as ctx:
        in_pool = ctx.enter_context(tc.tile_pool(name="in", bufs=4))
        it_pool = ctx.enter_context(tc.tile_pool(name="it", bufs=1))
        out_pool = ctx.enter_context(tc.tile_pool(name="out", bufs=3))

        # one-time item-space prepass over ALL items:
        #   pc_all = par * cnt (per-aspect broadcast), recip_all = 1/max(cnt+n,1)
        par_all = it_pool.tile([P, tot_icols * A], F32, tag="par")
        nc.sync.dma_start(par_all[:], d_par[:, :])
        cnt_all = it_pool.tile([P, tot_icols], F32, tag="cnt")
        nc.sync.dma_start(cnt_all[:], d_cnt[:, :])
        n_all = it_pool.tile([P, tot_icols], F32, tag="nn")
        nc.sync.dma_start(n_all[:], d_n[:, :])
        recip_all = it_pool.tile([P, tot_icols], F32, tag="recip")
        nc.vector.tensor_tensor(recip_all[:], cnt_all[:], n_all[:],
                                op=mybir.AluOpType.add)
        nc.vector.tensor_scalar_max(recip_all[:], recip_all[:], 1.0)
        nc.vector.reciprocal(recip_all[:], recip_all[:])
        nc.vector.tensor_tensor(
            par_all[:].rearrange("p (q a) -> p q a", a=A),
            par_all[:].rearrange("p (q a) -> p q a", a=A),
            cnt_all[:].unsqueeze(2).broadcast_to([P, tot_icols, A]),
            op=mybir.AluOpType.mult)
        pc_all = par_all

        for ci, q0, r, nq, scol0, icol0, merged in _schedule_order(_chunks_of(layout)):
            m = nq * r
            w = nq * A                                    # elements per slot-plane
            rat = in_pool.tile([P, m * A], F32, tag="rat")
            nc.sync.dma_start(rat[:], d_rat[:, scol0 * A:(scol0 + m) * A])
            wts = in_pool.tile([P, m * A], F32, tag="wts")
            nc.sync.dma_start(wts[:], d_wts[:, scol0 * A:(scol0 + m) * A])

            # segment sums: in-place tree of contiguous adds over slot-planes
            k = r
            while k > 1:
                h = k // 2
                nc.vector.tensor_tensor(
                    rat[:, 0:h * w], rat[:, 0:h * w],
                    rat[:, (k - h) * w:k * w], op=mybir.AluOpType.add)
                k -= h
            ssum = rat[:, 0:w]

            # work = clip((pc + ssum) * recip, 1, 5), in place in pc_all
            work = pc_all[:, icol0 * A:(icol0 + nq) * A]
            nc.vector.tensor_tensor(work, work, ssum, op=mybir.AluOpType.add)
            nc.vector.tensor_tensor(
                work.rearrange("p (q a) -> p q a", a=A),
                work.rearrange("p (q a) -> p q a", a=A),
                recip_all[:, icol0:icol0 + nq]
                    .unsqueeze(2).broadcast_to([P, nq, A]),
                op=mybir.AluOpType.mult)
            nc.vector.tensor_scalar(work, work, MIN_R, MAX_R,
                                    op0=mybir.AluOpType.max,
                                    op1=mybir.AluOpType.min)

            # wts *= work (broadcast profiles over slot-planes)
            if r > 1:
                nc.vector.tensor_tensor(
                    wts[:].rearrange("p (r qa) -> p r qa", r=r),
                    wts[:].rearrange("p (r qa) -> p r qa", r=r),
                    work.unsqueeze(1).broadcast_to([P, r, w]),
                    op=mybir.AluOpType.mult)
            else:
                nc.vector.tensor_tensor(wts[:], wts[:], work,
                                        op=mybir.AluOpType.mult)

            pred = out_pool.tile([P, m], F32, tag="pred")
            nc.vector.tensor_reduce(
                pred[:].rearrange("p (lq) -> p lq", lq=m),
                wts[:].rearrange("p (lq a) -> p lq a", lq=m, a=A),
                axis=mybir.AxisListType.X, op=mybir.AluOpType.add)
            nc.scalar.dma_start(d_out[:, scol0:scol0 + m], pred[:])

    nc.compile()
    return nc


def _get_program(layout, tot_icols, tot_cols):
    key = (layout, tot_icols, tot_cols)
    if key not in _program_cache:
        _program_cache[key] = _build_program(layout, tot_icols, tot_cols)
    return _program_cache[key]


def _prepare(items_parameters, items_counters, I_ids, A_weights, A_ratings):
    rt = _route(np.asarray(I_ids), items_parameters.shape[0])
    ti, tc_ = rt["tot_icols"], rt["tot_cols"]

    rat_s = np.zeros((NC, P, tc_, A), np.float32)
    wts_s = np.zeros((NC, P, tc_, A), np.float32)
    rat_s[rt["core_t"], rt["p_t"], rt["col_t"]] = np.asarray(A_ratings)[rt["order"]]
    wts_s[rt["core_t"], rt["p_t"], rt["col_t"]] = np.asarray(A_weights)[rt["order"]]
    par_s = np.zeros((NC, P, ti, A), np.float32)
    cnt_s = np.zeros((NC, P, ti), np.float32)
    n_s = np.zeros((NC, P, ti), np.float32)
    par_s[rt["core_j"], rt["p_j"], rt["item_col"]] = np.asarray(items_parameters)[rt["item_ids"]]
    cnt_s[rt["core_j"], rt["p_j"], rt["item_col"]] = np.asarray(items_counters)[rt["item_ids"]]
    n_s[rt["core_j"], rt["p_j"], rt["item_col"]] = rt["item_r"].astype(np.float32)

    in_maps = [dict(rat_s=rat_s[c].reshape(P, tc_ * A),
                    wts_s=wts_s[c].reshape(P, tc_ * A),
                    par_s=par_s[c].reshape(P, ti * A),
                    cnt_s=cnt_s[c].reshape(P, ti),
                    n_s=n_s[c].reshape(P, ti))
               for c in range(NC)]
    return rt, in_maps


def _run(inputs, trace=False, **kw):
    rt, in_maps = _prepare(**inputs)
    nc = _get_program(rt["layout"], rt["tot_icols"], rt["tot_cols"])
    res = run_bass_kernel_spmd(nc, in_maps, core_ids=list(range(NC)),
                               trace=trace, **kw)
    preds_s = np.stack([res.results[c]["preds_s"] for c in range(NC)])
    B = rt["order"].shape[0]
    out = np.empty(B, np.float32)
    out[rt["order"]] = preds_s[rt["core_t"], rt["p_t"], rt["col_t"]]
    return out, res


def kernel(items_parameters, items_counters, I_ids, A_weights, A_ratings):
    out, _ = _run(dict(items_parameters=items_parameters,
                       items_counters=items_counters,
                       I_ids=I_ids,
                       A_weights=A_weights,
                       A_ratings=A_ratings))
    return out
